# revision 1
# baseline (speedup 1.0000x reference)
"""Trainium2 Bass kernel for GroundwaterModel Jacobi pseudo-timestepping.

Solves 100 Jacobi steps of -div(exp(u) grad p) = f on a [1024,1024] grid,
sharded row-wise (x) across 8 NeuronCores with a 1-row halo exchange per
step (AllGather of pre-weighted boundary rows).

Math: with D = 2*eu + eu_xm + eu_ym (Jacobi diagonal), substitute
q = sqrt(D) * p.  The update becomes

  q'[i,j] = bx[i,j] q[i+1,j] + bx[i-1,j] q[i-1,j]
          + by[i,j] q[i,j+1] + by[i,j-1] q[i,j-1] + c[i,j]

with bx = eu/(s * s_up), by = eu/(s * s_yp), s = sqrt(D), c = h^2 f / s
(+ Dirichlet fold at the two y-boundary columns).  Every term is a pure
shift of an elementwise product, so the PE accumulates the whole update
into PSUM via shift/identity matmuls while the DVE only computes the four
products u1=bxd*q, u2=bx*q, u3=byd*q, u4=by*q.  Neumann x-edges are folded
into per-core shift-matrix corner entries; the inter-core halo is the
exchange of u1[row 0] (up) and u2[row 127] (down), selected from the
AllGather result by a per-core K=16 selection matmul.
"""

import numpy as np

GRID = 1024
NCORES = 8
P = 128          # rows per core = SBUF partitions
W = GRID - 2     # computed interior columns j=1..GRID-2
TS = 100

_cached = {}


def _host_inputs(u, f, n_cores, time_steps):
    """Per-core input dicts. All per-core variation lives in data."""
    N = u.shape[0]
    h = 1.0 / (N - 1)
    rows = N // n_cores
    xs = (np.arange(N, dtype=np.float64) * h).astype(np.float32)

    def clip_rows(lo):
        idx = np.clip(np.arange(lo, lo + rows), 0, N - 1)
        return u[idx].astype(np.float32)

    in_maps = []
    for c in range(n_cores):
        r0 = c * rows
        u0 = u[r0:r0 + rows].astype(np.float32)
        uu = clip_rows(r0 + 1)
        ud = clip_rows(r0 - 1)
        udd = clip_rows(r0 - 2)
        umid = u0.copy()
        if c == n_cores - 1:
            umid[-1] = u[N - 2]          # so denom_up[last] == denom[N-1]
        f0 = f[r0:r0 + rows].astype(np.float32)
        bc0 = xs[r0:r0 + rows].reshape(rows, 1).copy()
        bc1 = (1.0 - xs[r0:r0 + rows]).reshape(rows, 1).copy()

        sup = np.zeros((rows, rows), dtype=np.float32)
        for i in range(rows - 1):
            sup[i, i + 1] = 1.0
        if c == 0:
            sup[0, 0] = 1.0              # Neumann bottom edge via u1[0]
        sdn = np.zeros((rows, rows), dtype=np.float32)
        for i in range(1, rows):
            sdn[i, i - 1] = 1.0
        if c == n_cores - 1:
            sdn[rows - 1, rows - 1] = 1.0  # Neumann top edge via u2[last]

        et = np.zeros((2 * n_cores, rows), dtype=np.float32)
        if c > 0:
            et[2 * c - 1, 0] = 1.0       # prev core's u2[last] -> my row 0
        if c < n_cores - 1:
            et[2 * c + 2, rows - 1] = 1.0  # next core's u1[0] -> my last row
        in_maps.append({
            "u0": u0, "uu": uu, "ud": ud, "udd": udd, "umid": umid,
            "f0": f0, "bc0": bc0, "bc1": bc1,
            "supT": sup.T.copy(), "sdnT": sdn.T.copy(), "eT": et,
        })
    return in_maps


def _build(n_cores, time_steps, nx, ny):
    import concourse.bass as bass
    import concourse.bacc as bacc
    import concourse.mybir as mybir
    from concourse.tile import TileContext

    f32 = mybir.dt.float32
    f32r = mybir.dt.float32r
    AF = mybir.ActivationFunctionType
    OP = mybir.AluOpType
    G = ny
    Wl = G - 2
    h = 1.0 / (nx - 1)
    rows = nx // n_cores
    GR = 2 * n_cores                     # gathered rows

    nc = bacc.Bacc(
        "TRN2",
        target_bir_lowering=False,
        debug=False,
        num_devices=n_cores,
    )
    dp = nc.declare_dram_parameter
    u0_d = dp("u0", [rows, G], f32, isOutput=False)
    uu_d = dp("uu", [rows, G], f32, isOutput=False)
    ud_d = dp("ud", [rows, G], f32, isOutput=False)
    udd_d = dp("udd", [rows, G], f32, isOutput=False)
    umid_d = dp("umid", [rows, G], f32, isOutput=False)
    f0_d = dp("f0", [rows, G], f32, isOutput=False)
    bc0_d = dp("bc0", [rows, 1], f32, isOutput=False)
    bc1_d = dp("bc1", [rows, 1], f32, isOutput=False)
    supT_d = dp("supT", [rows, rows], f32, isOutput=False)
    sdnT_d = dp("sdnT", [rows, rows], f32, isOutput=False)
    eT_d = dp("eT", [GR, rows], f32, isOutput=False)
    pout_d = dp("pout", [rows, Wl], f32, isOutput=True)

    with TileContext(nc) as tc:
        with (
            tc.tile_pool(name="coef", bufs=1) as coef,
            tc.tile_pool(name="wts", bufs=1) as wts,
            tc.tile_pool(name="work", bufs=2) as work,
            tc.tile_pool(name="qp", bufs=2, space="PSUM") as qp,
            tc.tile_pool(name="dramp", bufs=2, space="DRAM") as dramp,
        ):
            # ---- persistent tiles ----
            bx = coef.tile([rows, Wl], f32, name="bx")
            bxd = coef.tile([rows, Wl], f32, name="bxd")
            by = coef.tile([rows, Wl], f32, name="by")
            cp = coef.tile([rows, Wl], f32, name="cp")
            cp0 = coef.tile([rows, Wl], f32, name="cp0")
            rs = coef.tile([rows, G], f32, name="rs")
            supT = wts.tile([rows, rows], f32, name="supT_t")
            sdnT = wts.tile([rows, rows], f32, name="sdnT_t")
            eT = wts.tile([GR, rows], f32, name="eT_t")
            nc.sync.dma_start(out=supT[:, :], in_=supT_d[:, :])
            nc.sync.dma_start(out=sdnT[:, :], in_=sdnT_d[:, :])
            nc.sync.dma_start(out=eT[:, :], in_=eT_d[:, :])

            # ---- setup: coefficients ----
            with tc.tile_pool(name="setup", bufs=1) as sp:
                u0 = sp.tile([rows, G], f32, name="u0_t")
                uu = sp.tile([rows, G], f32, name="uu_t")
                ud = sp.tile([rows, G], f32, name="ud_t")
                udd = sp.tile([rows, G], f32, name="udd_t")
                umid = sp.tile([rows, G], f32, name="umid_t")
                f0 = sp.tile([rows, G], f32, name="f0_t")
                bc0 = sp.tile([rows, 1], f32, name="bc0_t")
                bc1 = sp.tile([rows, 1], f32, name="bc1_t")
                nc.sync.dma_start(out=u0[:, :], in_=u0_d[:, :])
                nc.sync.dma_start(out=uu[:, :], in_=uu_d[:, :])
                nc.sync.dma_start(out=ud[:, :], in_=ud_d[:, :])
                nc.sync.dma_start(out=udd[:, :], in_=udd_d[:, :])
                nc.sync.dma_start(out=umid[:, :], in_=umid_d[:, :])
                nc.sync.dma_start(out=f0[:, :], in_=f0_d[:, :])
                nc.sync.dma_start(out=bc0[:, :], in_=bc0_d[:, :])
                nc.sync.dma_start(out=bc1[:, :], in_=bc1_d[:, :])

                eu = sp.tile([rows, G], f32, name="eu")
                eu_u = sp.tile([rows, G], f32, name="eu_u")
                eu_d = sp.tile([rows, G], f32, name="eu_d")
                eu_dd = sp.tile([rows, G], f32, name="eu_dd")
                eu_m = sp.tile([rows, G], f32, name="eu_m")
                nc.scalar.activation(eu[:, :], u0[:, :], AF.Exp)
                nc.scalar.activation(eu_u[:, :], uu[:, :], AF.Exp)
                nc.scalar.activation(eu_d[:, :], ud[:, :], AF.Exp)
                nc.scalar.activation(eu_dd[:, :], udd[:, :], AF.Exp)
                nc.scalar.activation(eu_m[:, :], umid[:, :], AF.Exp)

                den_s = sp.tile([rows, G], f32, name="den_s")
                den = sp.tile([rows, G], f32, name="den")
                dup_s = sp.tile([rows, G], f32, name="dup_s")
                dup = sp.tile([rows, G], f32, name="dup")
                ddn_s = sp.tile([rows, G], f32, name="ddn_s")
                ddn = sp.tile([rows, G], f32, name="ddn")
                V = nc.vector
                # denom cols 1..G-1
                V.scalar_tensor_tensor(den_s[:, 1:G], eu[:, 1:G], 2.0,
                                       eu_d[:, 1:G], OP.mult, OP.add)
                V.tensor_add(den[:, 1:G], den_s[:, 1:G], eu[:, 0:G - 1])
                V.scalar_tensor_tensor(dup_s[:, 1:G], eu_u[:, 1:G], 2.0,
                                       eu_m[:, 1:G], OP.mult, OP.add)
                V.tensor_add(dup[:, 1:G], dup_s[:, 1:G], eu_u[:, 0:G - 1])
                V.scalar_tensor_tensor(ddn_s[:, 1:G], eu_d[:, 1:G], 2.0,
                                       eu_dd[:, 1:G], OP.mult, OP.add)
                V.tensor_add(ddn[:, 1:G], ddn_s[:, 1:G], eu_d[:, 0:G - 1])

                rs_up = sp.tile([rows, G], f32, name="rs_up")
                rs_dn = sp.tile([rows, G], f32, name="rs_dn")
                nt_a = sp.tile([rows, G], f32, name="nt_a")
                nt_b = sp.tile([rows, G], f32, name="nt_b")

                def rsqrt_ref(out_ap, x_ap):
                    # ACT Sqrt seed + reciprocal, then 2 Newton iterations
                    # y' = y*(1.5 - 0.5*x*y^2) in fp32 on DVE.
                    nc.scalar.activation(nt_a[:, 1:G], x_ap, AF.Sqrt)
                    nc.vector.reciprocal(out_ap, nt_a[:, 1:G])
                    for _ in range(2):
                        nc.vector.tensor_mul(nt_a[:, 1:G], out_ap, out_ap)
                        nc.vector.tensor_mul(nt_b[:, 1:G], nt_a[:, 1:G], x_ap)
                        nc.vector.tensor_scalar(nt_a[:, 1:G], nt_b[:, 1:G],
                                                -0.5, 1.5, OP.mult, OP.add)
                        nc.vector.tensor_mul(nt_b[:, 1:G], out_ap,
                                             nt_a[:, 1:G])
                        nc.vector.tensor_copy(out_ap, nt_b[:, 1:G])
                    return out_ap

                rsqrt_ref(rs[:, 1:G], den[:, 1:G])
                rsqrt_ref(rs_up[:, 1:G], dup[:, 1:G])
                rsqrt_ref(rs_dn[:, 1:G], ddn[:, 1:G])

                t1 = sp.tile([rows, Wl], f32, name="t1")
                # bx[k] = eu[k+1]*rs[k+1]*rs_up[k+1]  (tile col k == global j=k+1)
                V.tensor_mul(t1[:, :], eu[:, 1:1 + Wl], rs[:, 1:1 + Wl])
                V.tensor_mul(bx[:, :], t1[:, :], rs_up[:, 1:1 + Wl])
                # bxd[k] = eu_d[k+1]*rs_dn[k+1]*rs[k+1]
                V.tensor_mul(t1[:, :], eu_d[:, 1:1 + Wl], rs_dn[:, 1:1 + Wl])
                V.tensor_mul(bxd[:, :], t1[:, :], rs[:, 1:1 + Wl])
                # by[k] = eu[k+1]*rs[k+1]*rs[k+2]
                V.tensor_mul(t1[:, :], eu[:, 1:1 + Wl], rs[:, 2:2 + Wl])
                V.tensor_mul(by[:, :], t1[:, :], rs[:, 1:1 + Wl])

                # cp: h2f*rs with Dirichlet fold at cols 0 and Wl-1
                h2f = sp.tile([rows, G], f32, name="h2f")
                V.tensor_scalar_mul(h2f[:, :], f0[:, :], h * h)
                e0 = sp.tile([rows, 1], f32, name="e0")
                e1 = sp.tile([rows, 1], f32, name="e1")
                V.scalar_tensor_tensor(e0[:, :], eu[:, 0:1], bc0[:, :],
                                       h2f[:, 1:2], OP.mult, OP.add)
                V.scalar_tensor_tensor(e1[:, :], eu[:, G - 2:G - 1], bc1[:, :],
                                       h2f[:, G - 2:G - 1], OP.mult, OP.add)
                V.tensor_mul(cp[:, 1:Wl - 1], h2f[:, 2:G - 2], rs[:, 2:G - 2])
                V.tensor_mul(cp[:, 0:1], e0[:, :], rs[:, 1:2])
                V.tensor_mul(cp[:, Wl - 1:Wl], e1[:, :], rs[:, G - 2:G - 1])
                # q_1 uses the unfolded constant (reference's p0 has zero BCs)
                V.tensor_mul(cp0[:, :], h2f[:, 1:G - 1], rs[:, 1:G - 1])

            # ---- iteration ----
            B0 = 512                     # PSUM bank split
            banks = [(0, min(B0, Wl))] + ([(B0, Wl)] if Wl > B0 else [])
            rg = [list(range(n_cores))]
            V = nc.vector
            mm = nc.tensor.matmul
            qf = None
            u1 = u2 = gsb = None
            for t in range(1, time_steps + 1):
                if t == 1:
                    qf = work.tile([rows, Wl], f32, tag="qf", name="qf_1")
                    V.tensor_copy(qf[:, :], cp0[:, :])  # q_1 = h2f*rs
                else:
                    # PE: x-shift terms + halo into PSUM
                    ps = qp.tile([rows, Wl], f32, tag="ps", name=f"ps_{t}")
                    for lo, hi in banks:
                        mm(ps[:, lo:hi], supT[:, :], u1[:, lo:hi],
                           start=True, stop=False)
                        mm(ps[:, lo:hi], sdnT[:, :], u2[:, lo:hi],
                           start=False, stop=False)
                        mm(ps[:, lo:hi], eT[:, :], gsb[:, lo:hi],
                           start=False, stop=True)
                    # DVE: y-terms (free-dim shifted reads of qf_prev)
                    y3 = work.tile([rows, Wl], f32, tag="y3", name=f"y3_{t}")
                    y4 = work.tile([rows, Wl], f32, tag="y4", name=f"y4_{t}")
                    V.memset(y3[:, Wl - 1:Wl], 0.0)
                    V.memset(y4[:, 0:1], 0.0)
                    V.tensor_mul(y3[:, 0:Wl - 1], by[:, 0:Wl - 1],
                                 qf[:, 1:Wl])
                    V.tensor_mul(y4[:, 1:Wl], by[:, 0:Wl - 1],
                                 qf[:, 0:Wl - 1])
                    a1 = work.tile([rows, Wl], f32, tag="a1", name=f"a1_{t}")
                    V.tensor_add(a1[:, :], y3[:, :], y4[:, :])
                    a2 = work.tile([rows, Wl], f32, tag="a2", name=f"a2_{t}")
                    V.tensor_add(a2[:, :], a1[:, :], cp[:, :])
                    qf = work.tile([rows, Wl], f32, tag="qf", name=f"qf_{t}")
                    V.tensor_add(qf[:, :], a2[:, :], ps[:, :])

                if t < time_steps:
                    u1 = work.tile([rows, Wl], f32, tag="u1", name=f"u1_{t}")
                    u2 = work.tile([rows, Wl], f32, tag="u2", name=f"u2_{t}")
                    V.tensor_mul(u1[:, :], bxd[:, :], qf[:, :])
                    V.tensor_mul(u2[:, :], bx[:, :], qf[:, :])
                    bounce = dramp.tile([2, Wl], f32, tag="bounce",
                                        name=f"bounce_{t}")
                    gath = dramp.tile([GR, Wl], f32, tag="gath",
                                      addr_space="Shared", name=f"gath_{t}")
                    nc.sync.dma_start(out=bounce[0:1, :], in_=u1[0:1, :])
                    nc.sync.dma_start(out=bounce[1:2, :],
                                      in_=u2[rows - 1:rows, :])
                    nc.gpsimd.collective_compute(
                        "AllGather", OP.bypass,
                        ins=[bounce.opt()], outs=[gath.opt()],
                        replica_groups=rg,
                    )
                    gsb = work.tile([GR, Wl], f32, tag="gsb", name=f"gsb_{t}")
                    nc.sync.dma_start(out=gsb[:, :], in_=gath[:, :])

            out_sb = coef.tile([rows, Wl], f32, name="out_sb")
            nc.vector.tensor_mul(out_sb[:, :], qf[:, :], rs[:, 1:1 + Wl])
            nc.sync.dma_start(out=pout_d[:, :], in_=out_sb[:, :])

    nc.finalize()
    return nc


def _get_nc(n_cores, time_steps, nx, ny):
    key = (n_cores, time_steps, nx, ny)
    if key not in _cached:
        _cached[key] = _build(n_cores, time_steps, nx, ny)
    return _cached[key]


def kernel(u, f, time_steps):
    from concourse.bass_utils import run_bass_kernel_spmd

    u = np.asarray(u)
    f = np.asarray(f)
    ts = int(time_steps)
    N = u.shape[0]
    n_cores = NCORES
    nc = _get_nc(n_cores, ts, N, u.shape[1])
    in_maps = _host_inputs(u, f, n_cores, ts)
    res = run_bass_kernel_spmd(nc, in_maps, list(range(n_cores))).results
    interior = np.concatenate([r["pout"] for r in res], axis=0)
    h = 1.0 / (N - 1)
    xs = (np.arange(N, dtype=np.float64) * h).astype(np.float32)
    out = np.empty((N, N), dtype=np.float32)
    out[:, 1:N - 1] = interior
    out[:, 0] = xs
    out[:, N - 1] = 1.0 - xs
    return out



# revision 3
# speedup vs baseline: 2.0568x; 2.0568x over previous
"""Trainium2 Bass kernel for GroundwaterModel Jacobi pseudo-timestepping.

100 Jacobi steps of -div(exp(u) grad p) = f on a [1024,1024] grid, with the
symmetrizing substitution q = sqrt(D) p (D = Jacobi diagonal), so the update is

  q'[i,j] = bxu[i,j] q[i+1,j] + bxd[i,j] q[i-1,j]
          + byu[i,j] q[i,j+1] + byd[i,j] q[i,j-1] + c[i,j]

Sharding: columns across 8 cores (128 cols each), rows chunked into the
partition dim: partition p holds rows 8p..8p+7, free dim = 8 row-windows of
CW = 128 + 2*GW columns (GW ghost columns each side).  With this layout both
x- and y-shifts are free-dim offsets applied by PE identity matmuls into PSUM
(plus 4 tiny cross-partition shift matmuls for row-chunk boundaries), the DVE
only computes the 4 coefficient products, and the ACT engine evicts PSUM.
Ghost columns are updated redundantly each step, so the halo AllGather runs
only every GW steps and stays off the per-step critical path.  fp16 datapath
(PSUM accumulation in fp32).
"""

import numpy as np

N = 1024
NCORES = 8
RP = 8               # rows per partition chunk
PART = 128           # partitions
MC = 128             # main cols per core
GW = 12              # ghost width == exchange period
CW = MC + 2 * GW     # stored cols per row-window (152)
F = RP * CW          # free size (1216)
TS = 100
BANKS = [(0, 512), (512, 1024), (1024, F)]

_cached = {}


def _coeff_full(u, f):
    """Full-grid float64 coefficient arrays."""
    u = np.asarray(u, np.float64)
    f = np.asarray(f, np.float64)
    h = 1.0 / (N - 1)
    eu = np.exp(u)
    eu_xm = np.concatenate([eu[:1, :], eu[:-1, :]], axis=0)
    eu_ym = np.concatenate([eu[:, :1], eu[:, :-1]], axis=1)
    den = 2.0 * eu + eu_xm + eu_ym
    s = np.sqrt(den)
    rs = 1.0 / s
    s_xp = np.concatenate([s[1:, :], s[-1:, :]], axis=0)
    s_xm = np.concatenate([s[:1, :], s[:-1, :]], axis=0)
    s_yp = np.concatenate([s[:, 1:], s[:, -1:]], axis=1)
    s_ym = np.concatenate([s[:, :1], s[:, :-1]], axis=1)
    # edge-replicated s_xp/s_xm make rows 0/1023 the Neumann self-terms
    bxu = eu / (s * s_xp)
    bxd = eu_xm / (s * s_xm)
    byu = eu / (s * s_yp)
    byd = eu_ym / (s * s_ym)
    cp = h * h * f * rs
    # Dirichlet columns: dead cells holding q = s*bc from step 1 on.
    # Zeroing follows the product-shift consumption pattern:
    #   col j's y-up   arrives via byd[j+1] (tensor C1 = byd*q read at +1)
    #   col j's y-down arrives via byu[j-1] (tensor D1 = byu*q read at -1)
    #   col j's x-up/down arrive via bxd[.,j]/bxu[.,j] (partition shifts)
    xs = np.arange(N, dtype=np.float64) * h
    cp[:, 0] = s[:, 0] * xs
    cp[:, N - 1] = s[:, N - 1] * (1.0 - xs)
    bxu[:, 0] = bxd[:, 0] = 0.0       # kill col 0 x-terms
    bxu[:, N - 1] = bxd[:, N - 1] = 0.0
    byd[:, 0] = 0.0                   # feeds only a dead ghost; keep clean
    byd[:, 1] = 0.0                   # col 0 must not receive y-up
    byu[:, N - 2] = 0.0               # col N-1 must not receive y-down
    byu[:, N - 1] = 0.0               # feeds only a dead ghost
    # byu[:, 0] and byd[:, N-1] KEEP true values: they supply col 1's y-down
    # and col N-2's y-up respectively.
    return bxu, bxd, byu, byd, cp, rs


def _tile_of(full, c, dtype=np.float16):
    """[1024,1024] full-grid array -> per-core [128, RP, CW] tile."""
    out = np.zeros((PART, RP, CW), np.float64)
    jj = np.arange(CW) + MC * c - GW
    valid = (jj >= 0) & (jj < N)
    out[:, :, valid] = full.reshape(PART, RP, N)[:, :, jj[valid]]
    return out.astype(dtype)


def _host_inputs(u, f):
    bxu, bxd, byu, byd, cp, rs = _coeff_full(u, f)

    ident = np.eye(PART, dtype=np.float16)
    wup = np.zeros((PART, PART), np.float16)   # out p <- in p+1
    for p in range(PART - 1):
        wup[p + 1, p] = 1.0
    wdn = np.zeros((PART, PART), np.float16)   # out p <- in p-1
    for p in range(1, PART):
        wdn[p - 1, p] = 1.0
    wc0 = np.zeros((PART, PART), np.float16)
    wc0[0, 0] = 1.0
    wc127 = np.zeros((PART, PART), np.float16)
    wc127[127, 127] = 1.0

    in_maps = []
    for c in range(NCORES):
        m = np.zeros((PART, NCORES, RP * 2 * GW), np.float16)
        sel = np.zeros((NCORES, RP, 2 * GW), np.float16)
        if c > 0:
            sel[c - 1, :, GW:2 * GW] = 1.0   # left neighbor's right-send
        if c < NCORES - 1:
            sel[c + 1, :, 0:GW] = 1.0        # right neighbor's left-send
        m[:, :, :] = sel.reshape(NCORES, RP * 2 * GW)[None, :, :]
        in_maps.append({
            "bxdT": _tile_of(bxd, c),
            "bxuT": _tile_of(bxu, c),
            "bydT": _tile_of(byd, c),
            "byuT": _tile_of(byu, c),
            "cpT": _tile_of(cp, c),
            "rsT": _tile_of(rs, c, np.float32)[:, :, GW:GW + MC].copy(),
            "mask": m,
            "wI": ident, "wUp": wup, "wDn": wdn,
            "wC0": wc0, "wC127": wc127,
        })
    return in_maps


def _build():
    import concourse.bass as bass
    import concourse.bacc as bacc
    import concourse.mybir as mybir
    from concourse.tile import TileContext

    f32 = mybir.dt.float32
    f16 = mybir.dt.float16
    AF = mybir.ActivationFunctionType
    OP = mybir.AluOpType
    SG = RP * 2 * GW                     # send payload elems per partition

    nc = bacc.Bacc("TRN2", target_bir_lowering=False, debug=False,
                   num_devices=NCORES)
    dp = nc.declare_dram_parameter
    bxd_d = dp("bxdT", [PART, RP, CW], f16, isOutput=False)
    bxu_d = dp("bxuT", [PART, RP, CW], f16, isOutput=False)
    byd_d = dp("bydT", [PART, RP, CW], f16, isOutput=False)
    byu_d = dp("byuT", [PART, RP, CW], f16, isOutput=False)
    cp_d = dp("cpT", [PART, RP, CW], f16, isOutput=False)
    rs_d = dp("rsT", [PART, RP, MC], f32, isOutput=False)
    mask_d = dp("mask", [PART, NCORES, SG], f16, isOutput=False)
    w_ds = {nm: dp(nm, [PART, PART], f16, isOutput=False)
            for nm in ("wI", "wUp", "wDn", "wC0", "wC127")}
    pout_d = dp("pout", [PART, RP * MC], f32, isOutput=True)

    with TileContext(nc) as tc:
        with (
            tc.tile_pool(name="coef", bufs=1) as coef,
            tc.tile_pool(name="work", bufs=2) as work,
            tc.tile_pool(name="qp", bufs=2, space="PSUM") as qp,
            tc.tile_pool(name="dramp", bufs=2, space="DRAM") as dramp,
        ):
            bxdT = coef.tile([PART, RP, CW], f16, name="bxdT_t")
            bxuT = coef.tile([PART, RP, CW], f16, name="bxuT_t")
            bydT = coef.tile([PART, RP, CW], f16, name="bydT_t")
            byuT = coef.tile([PART, RP, CW], f16, name="byuT_t")
            cpT = coef.tile([PART, RP, CW], f16, name="cpT_t")
            rsT = coef.tile([PART, RP, MC], f32, name="rsT_t")
            mask = coef.tile([PART, NCORES, SG], f16, name="mask_t")
            ws = {nm: coef.tile([PART, PART], f16, name=f"{nm}_t")
                  for nm in w_ds}
            nc.sync.dma_start(out=bxdT[:, :, :], in_=bxd_d[:, :, :])
            nc.sync.dma_start(out=bxuT[:, :, :], in_=bxu_d[:, :, :])
            nc.sync.dma_start(out=bydT[:, :, :], in_=byd_d[:, :, :])
            nc.sync.dma_start(out=byuT[:, :, :], in_=byu_d[:, :, :])
            nc.sync.dma_start(out=cpT[:, :, :], in_=cp_d[:, :, :])
            nc.sync.dma_start(out=rsT[:, :, :], in_=rs_d[:, :, :])
            nc.sync.dma_start(out=mask[:, :, :], in_=mask_d[:, :, :])
            for nm, d in w_ds.items():
                nc.sync.dma_start(out=ws[nm][:, :], in_=d[:, :])

            cpF = cpT.rearrange("p r c -> p (r c)")
            V = nc.vector
            mm = nc.tensor.matmul

            q = work.tile([PART, RP, CW], f16, tag="q", name="q_0")
            V.memset(q[:, :, :], 0.0)

            for t in range(1, TS + 1):
                A = work.tile([PART, RP, CW], f16, tag="A", name=f"A_{t}")
                B = work.tile([PART, RP, CW], f16, tag="B", name=f"B_{t}")
                C1 = work.tile([PART, RP, CW], f16, tag="C1", name=f"C1_{t}")
                D1 = work.tile([PART, RP, CW], f16, tag="D1", name=f"D1_{t}")
                V.tensor_mul(A[:, :, :], bxdT[:, :, :], q[:, :, :])
                V.tensor_mul(B[:, :, :], bxuT[:, :, :], q[:, :, :])
                V.tensor_mul(C1[:, :, :], bydT[:, :, :], q[:, :, :])
                V.tensor_mul(D1[:, :, :], byuT[:, :, :], q[:, :, :])
                Af = A.rearrange("p r c -> p (r c)")
                Bf = B.rearrange("p r c -> p (r c)")
                Cf = C1.rearrange("p r c -> p (r c)")
                Df = D1.rearrange("p r c -> p (r c)")

                ps = qp.tile([PART, 1536], f32, tag="ps", name=f"ps_{t}")
                for lo, hi in BANKS:
                    mms = [(ps[:, lo:hi], ws["wI"], cpF[:, lo:hi])]
                    # x-up: out x gets A[x+CW], valid out < 7*CW
                    a, b = lo, min(hi, 7 * CW)
                    if a < b:
                        mms.append((ps[:, a:b], ws["wI"], Af[:, a + CW:b + CW]))
                    # x-down: out x gets B[x-CW], valid out >= CW
                    a, b = max(lo, CW), hi
                    if a < b:
                        mms.append((ps[:, a:b], ws["wI"], Bf[:, a - CW:b - CW]))
                    # y-up: out x gets C1[x+1], valid out < F-1
                    a, b = lo, min(hi, F - 1)
                    if a < b:
                        mms.append((ps[:, a:b], ws["wI"], Cf[:, a + 1:b + 1]))
                    # y-down: out x gets D1[x-1], valid out >= 1
                    a, b = max(lo, 1), hi
                    if a < b:
                        mms.append((ps[:, a:b], ws["wI"], Df[:, a - 1:b - 1]))
                    if lo == 0:
                        # r=0 x-down from (p-1, r7); global row 0 self-term
                        mms.append((ps[:, 0:CW], ws["wDn"], Bf[:, 7 * CW:F]))
                        mms.append((ps[:, 0:CW], ws["wC0"], Af[:, 0:CW]))
                    if hi == F:
                        # r=7 x-up from (p+1, r0); global row 1023 self-term
                        mms.append((ps[:, 7 * CW:F], ws["wUp"], Af[:, 0:CW]))
                        mms.append((ps[:, 7 * CW:F], ws["wC127"], Bf[:, 7 * CW:F]))
                    for i, (o, w, x) in enumerate(mms):
                        mm(o, w, x, start=(i == 0), stop=(i == len(mms) - 1))

                qn = work.tile([PART, RP, CW], f16, tag="q", name=f"q_{t}")
                qnF = qn.rearrange("p r c -> p (r c)")
                nc.scalar.activation(qnF[:, 0:F], ps[:, 0:F], AF.Copy)
                q = qn

                if t % GW == 0 and t < TS:
                    stage = work.tile([PART, RP, 2 * GW], f16, tag="stage",
                                      name=f"stage_{t}")
                    V.tensor_copy(stage[:, :, 0:GW], q[:, :, GW:2 * GW])
                    V.tensor_copy(stage[:, :, GW:2 * GW], q[:, :, MC:MC + GW])
                    bounce = dramp.tile([PART, SG], f16, tag="bounce",
                                        name=f"bounce_{t}")
                    gath = dramp.tile([NCORES, PART, SG], f16, tag="gath",
                                      addr_space="Shared", name=f"gath_{t}")
                    stF = stage.rearrange("p r c -> p (r c)")
                    nc.sync.dma_start(out=bounce[:, :], in_=stF[:, :])
                    nc.gpsimd.collective_compute(
                        "AllGather", OP.bypass,
                        ins=[bounce.opt()], outs=[gath.opt()],
                        replica_groups=[list(range(NCORES))],
                    )
                    GG = work.tile([PART, NCORES, SG], f16, tag="GG",
                                   name=f"GG_{t}")
                    nc.sync.dma_start(out=GG[:, :, :],
                                      in_=gath[:, :, :].transpose([1, 0, 2]))
                    GGm = work.tile([PART, NCORES, SG], f16, tag="GGm",
                                    name=f"GGm_{t}")
                    V.tensor_mul(GGm[:, :, :], GG[:, :, :], mask[:, :, :])
                    T1 = work.tile([PART, 4, SG], f16, tag="T1", name=f"T1_{t}")
                    V.tensor_add(T1[:, :, :], GGm[:, 0:4, :], GGm[:, 4:8, :])
                    T2 = work.tile([PART, 2, SG], f16, tag="T2", name=f"T2_{t}")
                    V.tensor_add(T2[:, :, :], T1[:, 0:2, :], T1[:, 2:4, :])
                    R = work.tile([PART, 1, SG], f16, tag="R", name=f"R_{t}")
                    V.tensor_add(R[:, :, :], T2[:, 0:1, :], T2[:, 1:2, :])
                    Rv = R.rearrange("p a (r c) -> p (a r) c", c=2 * GW)
                    V.tensor_copy(q[:, :, 0:GW], Rv[:, :, GW:2 * GW])
                    V.tensor_copy(q[:, :, MC + GW:CW], Rv[:, :, 0:GW])

            outt = coef.tile([PART, RP, MC], f32, name="outt")
            V.tensor_mul(outt[:, :, :], q[:, :, GW:GW + MC], rsT[:, :, :])
            oF = outt.rearrange("p r c -> p (r c)")
            nc.sync.dma_start(out=pout_d[:, :], in_=oF[:, :])

    nc.finalize()
    return nc


def _get_nc():
    if "nc" not in _cached:
        _cached["nc"] = _build()
    return _cached["nc"]


def kernel(u, f, time_steps):
    from concourse.bass_utils import run_bass_kernel_spmd

    u = np.asarray(u)
    f = np.asarray(f)
    assert int(time_steps) == TS and u.shape == (N, N)
    nc = _get_nc()
    in_maps = _host_inputs(u, f)
    res = run_bass_kernel_spmd(nc, in_maps, list(range(NCORES))).results
    h = 1.0 / (N - 1)
    xs = (np.arange(N, dtype=np.float64) * h).astype(np.float32)
    out = np.empty((N, N), dtype=np.float32)
    for c in range(NCORES):
        blk = res[c]["pout"].reshape(PART, RP, MC).reshape(N, MC)
        out[:, MC * c:MC * (c + 1)] = blk
    out[:, 0] = xs
    out[:, N - 1] = 1.0 - xs
    return out


# revision 4
# speedup vs baseline: 2.5990x; 1.2636x over previous
"""Trainium2 Bass kernel for GroundwaterModel Jacobi pseudo-timestepping.

100 Jacobi steps of -div(exp(u) grad p) = f on a [1024,1024] grid, with the
symmetrizing substitution q = sqrt(D) p (D = Jacobi diagonal), so the update is

  q'[i,j] = bxu[i,j] q[i+1,j] + bxd[i,j] q[i-1,j]
          + byu[i,j] q[i,j+1] + byd[i,j] q[i,j-1] + c[i,j]

Sharding: columns across 8 cores (128 cols each), rows chunked into the
partition dim: partition p holds rows 8p..8p+7, free dim = 8 row-windows of
CW = 128 + 2*GW columns (GW ghost columns each side).  With this layout both
x- and y-shifts are free-dim offsets applied by PE identity matmuls into PSUM
(plus 4 tiny cross-partition shift matmuls for row-chunk boundaries), the DVE
only computes the 4 coefficient products, and the ACT engine evicts PSUM.
Ghost columns are updated redundantly each step, so the halo AllGather runs
only every GW steps and stays off the per-step critical path.  fp16 datapath
(PSUM accumulation in fp32).
"""

import numpy as np

N = 1024
NCORES = 8
RP = 8               # rows per partition chunk
PART = 128           # partitions
MC = 128             # main cols per core
GW = 12              # ghost width == exchange period
CW = MC + 2 * GW     # stored cols per row-window (152)
F = RP * CW          # free size (1216)
TS = 100
BANKS = [(0, 512), (512, 1024), (1024, F)]

_cached = {}


def _coeff_full(u, f):
    """Full-grid float64 coefficient arrays."""
    u = np.asarray(u, np.float64)
    f = np.asarray(f, np.float64)
    h = 1.0 / (N - 1)
    eu = np.exp(u)
    eu_xm = np.concatenate([eu[:1, :], eu[:-1, :]], axis=0)
    eu_ym = np.concatenate([eu[:, :1], eu[:, :-1]], axis=1)
    den = 2.0 * eu + eu_xm + eu_ym
    s = np.sqrt(den)
    rs = 1.0 / s
    s_xp = np.concatenate([s[1:, :], s[-1:, :]], axis=0)
    s_xm = np.concatenate([s[:1, :], s[:-1, :]], axis=0)
    s_yp = np.concatenate([s[:, 1:], s[:, -1:]], axis=1)
    s_ym = np.concatenate([s[:, :1], s[:, :-1]], axis=1)
    # edge-replicated s_xp/s_xm make rows 0/1023 the Neumann self-terms
    bxu = eu / (s * s_xp)
    bxd = eu_xm / (s * s_xm)
    byu = eu / (s * s_yp)
    byd = eu_ym / (s * s_ym)
    cp = h * h * f * rs
    # Dirichlet columns: dead cells holding q = s*bc from step 1 on.
    # Zeroing follows the product-shift consumption pattern:
    #   col j's y-up   arrives via byd[j+1] (tensor C1 = byd*q read at +1)
    #   col j's y-down arrives via byu[j-1] (tensor D1 = byu*q read at -1)
    #   col j's x-up/down arrive via bxd[.,j]/bxu[.,j] (partition shifts)
    xs = np.arange(N, dtype=np.float64) * h
    cp[:, 0] = s[:, 0] * xs
    cp[:, N - 1] = s[:, N - 1] * (1.0 - xs)
    bxu[:, 0] = bxd[:, 0] = 0.0       # kill col 0 x-terms
    bxu[:, N - 1] = bxd[:, N - 1] = 0.0
    byd[:, 0] = 0.0                   # feeds only a dead ghost; keep clean
    byd[:, 1] = 0.0                   # col 0 must not receive y-up
    byu[:, N - 2] = 0.0               # col N-1 must not receive y-down
    byu[:, N - 1] = 0.0               # feeds only a dead ghost
    # byu[:, 0] and byd[:, N-1] KEEP true values: they supply col 1's y-down
    # and col N-2's y-up respectively.
    return bxu, bxd, byu, byd, cp, rs


def _tile_of(full, c, dtype=np.float16):
    """[1024,1024] full-grid array -> per-core [128, RP, CW] tile."""
    out = np.zeros((PART, RP, CW), np.float64)
    jj = np.arange(CW) + MC * c - GW
    valid = (jj >= 0) & (jj < N)
    out[:, :, valid] = full.reshape(PART, RP, N)[:, :, jj[valid]]
    return out.astype(dtype)


def _host_inputs(u, f):
    bxu, bxd, byu, byd, cp, rs = _coeff_full(u, f)

    ident = np.eye(PART, dtype=np.float16)
    wup = np.zeros((PART, PART), np.float16)   # out p <- in p+1
    for p in range(PART - 1):
        wup[p + 1, p] = 1.0
    wdn = np.zeros((PART, PART), np.float16)   # out p <- in p-1
    for p in range(1, PART):
        wdn[p - 1, p] = 1.0
    wc0 = np.zeros((PART, PART), np.float16)
    wc0[0, 0] = 1.0
    wc127 = np.zeros((PART, PART), np.float16)
    wc127[127, 127] = 1.0

    in_maps = []
    for c in range(NCORES):
        m = np.zeros((PART, NCORES, RP * 2 * GW), np.float16)
        sel = np.zeros((NCORES, RP, 2 * GW), np.float16)
        if c > 0:
            sel[c - 1, :, GW:2 * GW] = 1.0   # left neighbor's right-send
        if c < NCORES - 1:
            sel[c + 1, :, 0:GW] = 1.0        # right neighbor's left-send
        m[:, :, :] = sel.reshape(NCORES, RP * 2 * GW)[None, :, :]
        in_maps.append({
            "bxdT": _tile_of(bxd, c),
            "bxuT": _tile_of(bxu, c),
            "bydT": _tile_of(byd, c),
            "byuT": _tile_of(byu, c),
            "cpT": _tile_of(cp, c),
            "rsT": _tile_of(rs, c, np.float32)[:, :, GW:GW + MC].copy(),
            "mask": m,
            "wI": ident, "wUp": wup, "wDn": wdn,
            "wC0": wc0, "wC127": wc127,
        })
    return in_maps


def _build():
    import concourse.bass as bass
    import concourse.bacc as bacc
    import concourse.mybir as mybir
    from concourse.tile import TileContext

    f32 = mybir.dt.float32
    f16 = mybir.dt.float16
    AF = mybir.ActivationFunctionType
    OP = mybir.AluOpType
    SG = RP * 2 * GW                     # send payload elems per partition

    nc = bacc.Bacc("TRN2", target_bir_lowering=False, debug=False,
                   num_devices=NCORES)
    dp = nc.declare_dram_parameter
    bxd_d = dp("bxdT", [PART, RP, CW], f16, isOutput=False)
    bxu_d = dp("bxuT", [PART, RP, CW], f16, isOutput=False)
    byd_d = dp("bydT", [PART, RP, CW], f16, isOutput=False)
    byu_d = dp("byuT", [PART, RP, CW], f16, isOutput=False)
    cp_d = dp("cpT", [PART, RP, CW], f16, isOutput=False)
    rs_d = dp("rsT", [PART, RP, MC], f32, isOutput=False)
    mask_d = dp("mask", [PART, NCORES, SG], f16, isOutput=False)
    w_ds = {nm: dp(nm, [PART, PART], f16, isOutput=False)
            for nm in ("wI", "wUp", "wDn", "wC0", "wC127")}
    pout_d = dp("pout", [PART, RP * MC], f32, isOutput=True)

    with TileContext(nc) as tc:
        with (
            tc.tile_pool(name="coef", bufs=1) as coef,
            tc.tile_pool(name="work", bufs=2) as work,
            tc.tile_pool(name="qp", bufs=2, space="PSUM") as qp,
            tc.tile_pool(name="dramp", bufs=2, space="DRAM") as dramp,
        ):
            bxdT = coef.tile([PART, RP, CW], f16, name="bxdT_t")
            bxuT = coef.tile([PART, RP, CW], f16, name="bxuT_t")
            bydT = coef.tile([PART, RP, CW], f16, name="bydT_t")
            byuT = coef.tile([PART, RP, CW], f16, name="byuT_t")
            cpT = coef.tile([PART, RP, CW], f16, name="cpT_t")
            rsT = coef.tile([PART, RP, MC], f32, name="rsT_t")
            mask = coef.tile([PART, NCORES, SG], f16, name="mask_t")
            ws = {nm: coef.tile([PART, PART], f16, name=f"{nm}_t")
                  for nm in w_ds}
            nc.sync.dma_start(out=bxdT[:, :, :], in_=bxd_d[:, :, :])
            nc.sync.dma_start(out=bxuT[:, :, :], in_=bxu_d[:, :, :])
            nc.sync.dma_start(out=bydT[:, :, :], in_=byd_d[:, :, :])
            nc.sync.dma_start(out=byuT[:, :, :], in_=byu_d[:, :, :])
            nc.sync.dma_start(out=cpT[:, :, :], in_=cp_d[:, :, :])
            nc.sync.dma_start(out=rsT[:, :, :], in_=rs_d[:, :, :])
            nc.sync.dma_start(out=mask[:, :, :], in_=mask_d[:, :, :])
            for nm, d in w_ds.items():
                nc.sync.dma_start(out=ws[nm][:, :], in_=d[:, :])

            cpF = cpT.rearrange("p r c -> p (r c)")
            V = nc.vector
            mm = nc.tensor.matmul

            q = work.tile([PART, RP, CW], f16, tag="q", name="q_0")
            V.memset(q[:, :, :], 0.0)

            for t in range(1, TS + 1):
                A = work.tile([PART, RP, CW], f16, tag="A", name=f"A_{t}")
                B = work.tile([PART, RP, CW], f16, tag="B", name=f"B_{t}")
                C1 = work.tile([PART, RP, CW], f16, tag="C1", name=f"C1_{t}")
                D1 = work.tile([PART, RP, CW], f16, tag="D1", name=f"D1_{t}")
                V.tensor_mul(A[:, :, :], bxdT[:, :, :], q[:, :, :])
                V.tensor_mul(B[:, :, :], bxuT[:, :, :], q[:, :, :])
                V.tensor_mul(C1[:, :, :], bydT[:, :, :], q[:, :, :])
                V.tensor_mul(D1[:, :, :], byuT[:, :, :], q[:, :, :])
                Af = A.rearrange("p r c -> p (r c)")
                Bf = B.rearrange("p r c -> p (r c)")
                Cf = C1.rearrange("p r c -> p (r c)")
                Df = D1.rearrange("p r c -> p (r c)")

                ps = qp.tile([PART, 1536], f32, tag="ps", name=f"ps_{t}")
                # Emit matmuls in global dependency order (PE executes its
                # queue in order; grouping by bank would stall PE on the last
                # product once per bank).  Per-bank accumulation groups:
                # cp opens (start=True), y-down closes (stop=True).
                for lo, hi in BANKS:
                    mm(ps[:, lo:hi], ws["wI"], cpF[:, lo:hi],
                       start=True, stop=False)
                for lo, hi in BANKS:
                    # x-up: out x gets A[x+CW], valid out < 7*CW
                    a, b = lo, min(hi, 7 * CW)
                    if a < b:
                        mm(ps[:, a:b], ws["wI"], Af[:, a + CW:b + CW],
                           start=False, stop=False)
                for lo, hi in BANKS:
                    # x-down: out x gets B[x-CW], valid out >= CW
                    a, b = max(lo, CW), hi
                    if a < b:
                        mm(ps[:, a:b], ws["wI"], Bf[:, a - CW:b - CW],
                           start=False, stop=False)
                # cross-partition row couplings (need A, B only)
                mm(ps[:, 0:CW], ws["wDn"], Bf[:, 7 * CW:F],
                   start=False, stop=False)
                mm(ps[:, 0:CW], ws["wC0"], Af[:, 0:CW],
                   start=False, stop=False)
                mm(ps[:, 7 * CW:F], ws["wUp"], Af[:, 0:CW],
                   start=False, stop=False)
                mm(ps[:, 7 * CW:F], ws["wC127"], Bf[:, 7 * CW:F],
                   start=False, stop=False)
                for lo, hi in BANKS:
                    # y-up: out x gets C1[x+1], valid out < F-1
                    a, b = lo, min(hi, F - 1)
                    if a < b:
                        mm(ps[:, a:b], ws["wI"], Cf[:, a + 1:b + 1],
                           start=False, stop=False)
                for lo, hi in BANKS:
                    # y-down closes each bank's accumulation group
                    a, b = max(lo, 1), hi
                    mm(ps[:, a:b], ws["wI"], Df[:, a - 1:b - 1],
                       start=False, stop=True)

                qn = work.tile([PART, RP, CW], f16, tag="q", name=f"q_{t}")
                qnF = qn.rearrange("p r c -> p (r c)")
                # bank-sliced eviction overlaps ACT with PE's remaining banks
                for lo, hi in BANKS:
                    nc.scalar.activation(qnF[:, lo:hi], ps[:, lo:hi], AF.Copy)
                q = qn

                if t % GW == 0 and t < TS:
                    stage = work.tile([PART, RP, 2 * GW], f16, tag="stage",
                                      name=f"stage_{t}")
                    V.tensor_copy(stage[:, :, 0:GW], q[:, :, GW:2 * GW])
                    V.tensor_copy(stage[:, :, GW:2 * GW], q[:, :, MC:MC + GW])
                    bounce = dramp.tile([PART, SG], f16, tag="bounce",
                                        name=f"bounce_{t}")
                    gath = dramp.tile([NCORES, PART, SG], f16, tag="gath",
                                      addr_space="Shared", name=f"gath_{t}")
                    stF = stage.rearrange("p r c -> p (r c)")
                    nc.sync.dma_start(out=bounce[:, :], in_=stF[:, :])
                    nc.gpsimd.collective_compute(
                        "AllGather", OP.bypass,
                        ins=[bounce.opt()], outs=[gath.opt()],
                        replica_groups=[list(range(NCORES))],
                    )
                    GG = work.tile([PART, NCORES, SG], f16, tag="GG",
                                   name=f"GG_{t}")
                    nc.sync.dma_start(out=GG[:, :, :],
                                      in_=gath[:, :, :].transpose([1, 0, 2]))
                    GGm = work.tile([PART, NCORES, SG], f16, tag="GGm",
                                    name=f"GGm_{t}")
                    V.tensor_mul(GGm[:, :, :], GG[:, :, :], mask[:, :, :])
                    T1 = work.tile([PART, 4, SG], f16, tag="T1", name=f"T1_{t}")
                    V.tensor_add(T1[:, :, :], GGm[:, 0:4, :], GGm[:, 4:8, :])
                    T2 = work.tile([PART, 2, SG], f16, tag="T2", name=f"T2_{t}")
                    V.tensor_add(T2[:, :, :], T1[:, 0:2, :], T1[:, 2:4, :])
                    R = work.tile([PART, 1, SG], f16, tag="R", name=f"R_{t}")
                    V.tensor_add(R[:, :, :], T2[:, 0:1, :], T2[:, 1:2, :])
                    Rv = R.rearrange("p a (r c) -> p (a r) c", c=2 * GW)
                    V.tensor_copy(q[:, :, 0:GW], Rv[:, :, GW:2 * GW])
                    V.tensor_copy(q[:, :, MC + GW:CW], Rv[:, :, 0:GW])

            outt = coef.tile([PART, RP, MC], f32, name="outt")
            V.tensor_mul(outt[:, :, :], q[:, :, GW:GW + MC], rsT[:, :, :])
            oF = outt.rearrange("p r c -> p (r c)")
            nc.sync.dma_start(out=pout_d[:, :], in_=oF[:, :])

    nc.finalize()
    return nc


def _get_nc():
    if "nc" not in _cached:
        _cached["nc"] = _build()
    return _cached["nc"]


def kernel(u, f, time_steps):
    from concourse.bass_utils import run_bass_kernel_spmd

    u = np.asarray(u)
    f = np.asarray(f)
    assert int(time_steps) == TS and u.shape == (N, N)
    nc = _get_nc()
    in_maps = _host_inputs(u, f)
    res = run_bass_kernel_spmd(nc, in_maps, list(range(NCORES))).results
    h = 1.0 / (N - 1)
    xs = (np.arange(N, dtype=np.float64) * h).astype(np.float32)
    out = np.empty((N, N), dtype=np.float32)
    for c in range(NCORES):
        blk = res[c]["pout"].reshape(PART, RP, MC).reshape(N, MC)
        out[:, MC * c:MC * (c + 1)] = blk
    out[:, 0] = xs
    out[:, N - 1] = 1.0 - xs
    return out


# revision 5
# speedup vs baseline: 2.7307x; 1.0507x over previous
"""Trainium2 Bass kernel for GroundwaterModel Jacobi pseudo-timestepping.

100 Jacobi steps of -div(exp(u) grad p) = f on a [1024,1024] grid, with the
symmetrizing substitution q = sqrt(D) p (D = Jacobi diagonal), so the update is

  q'[i,j] = bxu[i,j] q[i+1,j] + bxd[i,j] q[i-1,j]
          + byu[i,j] q[i,j+1] + byd[i,j] q[i,j-1] + c[i,j]

Sharding: columns across 8 cores (128 cols each), rows chunked into the
partition dim: partition p holds rows 8p..8p+7, free dim = 8 row-windows of
CW = 128 + 2*GW columns (GW ghost columns each side).  With this layout both
x- and y-shifts are free-dim offsets applied by PE identity matmuls into PSUM
(plus 4 tiny cross-partition shift matmuls for row-chunk boundaries), the DVE
only computes the 4 coefficient products, and the ACT engine evicts PSUM.
Ghost columns are updated redundantly each step, so the halo AllGather runs
only every GW steps and stays off the per-step critical path.  fp16 datapath
(PSUM accumulation in fp32).
"""

import numpy as np

N = 1024
NCORES = 8
RP = 8               # rows per partition chunk
PART = 128           # partitions
MC = 128             # main cols per core
GW = 12              # ghost width == exchange period
CW = MC + 2 * GW     # stored cols per row-window (152)
F = RP * CW          # free size (1216)
TS = 100
BANKS = [(0, 512), (512, 1024), (1024, F)]

_cached = {}


def _coeff_full(u, f):
    """Full-grid float64 coefficient arrays."""
    u = np.asarray(u, np.float64)
    f = np.asarray(f, np.float64)
    h = 1.0 / (N - 1)
    eu = np.exp(u)
    eu_xm = np.concatenate([eu[:1, :], eu[:-1, :]], axis=0)
    eu_ym = np.concatenate([eu[:, :1], eu[:, :-1]], axis=1)
    den = 2.0 * eu + eu_xm + eu_ym
    s = np.sqrt(den)
    rs = 1.0 / s
    s_xp = np.concatenate([s[1:, :], s[-1:, :]], axis=0)
    s_xm = np.concatenate([s[:1, :], s[:-1, :]], axis=0)
    s_yp = np.concatenate([s[:, 1:], s[:, -1:]], axis=1)
    s_ym = np.concatenate([s[:, :1], s[:, :-1]], axis=1)
    # edge-replicated s_xp/s_xm make rows 0/1023 the Neumann self-terms
    bxu = eu / (s * s_xp)
    bxd = eu_xm / (s * s_xm)
    byu = eu / (s * s_yp)
    byd = eu_ym / (s * s_ym)
    cp = h * h * f * rs
    # Dirichlet columns: dead cells holding q = s*bc from step 1 on.
    # Zeroing follows the product-shift consumption pattern:
    #   col j's y-up   arrives via byd[j+1] (tensor C1 = byd*q read at +1)
    #   col j's y-down arrives via byu[j-1] (tensor D1 = byu*q read at -1)
    #   col j's x-up/down arrive via bxd[.,j]/bxu[.,j] (partition shifts)
    xs = np.arange(N, dtype=np.float64) * h
    cp[:, 0] = s[:, 0] * xs
    cp[:, N - 1] = s[:, N - 1] * (1.0 - xs)
    bxu[:, 0] = bxd[:, 0] = 0.0       # kill col 0 x-terms
    bxu[:, N - 1] = bxd[:, N - 1] = 0.0
    byd[:, 0] = 0.0                   # feeds only a dead ghost; keep clean
    byd[:, 1] = 0.0                   # col 0 must not receive y-up
    byu[:, N - 2] = 0.0               # col N-1 must not receive y-down
    byu[:, N - 1] = 0.0               # feeds only a dead ghost
    # byu[:, 0] and byd[:, N-1] KEEP true values: they supply col 1's y-down
    # and col N-2's y-up respectively.
    return bxu, bxd, byu, byd, cp, rs


def _tile_of(full, c, dtype=np.float16):
    """[1024,1024] full-grid array -> per-core [128, RP, CW] tile."""
    out = np.zeros((PART, RP, CW), np.float64)
    jj = np.arange(CW) + MC * c - GW
    valid = (jj >= 0) & (jj < N)
    out[:, :, valid] = full.reshape(PART, RP, N)[:, :, jj[valid]]
    return out.astype(dtype)


def _host_inputs(u, f):
    bxu, bxd, byu, byd, cp, rs = _coeff_full(u, f)

    ident = np.eye(PART, dtype=np.float16)
    wup = np.zeros((PART, PART), np.float16)   # out p <- in p+1
    for p in range(PART - 1):
        wup[p + 1, p] = 1.0
    wdn = np.zeros((PART, PART), np.float16)   # out p <- in p-1
    for p in range(1, PART):
        wdn[p - 1, p] = 1.0
    wc0 = np.zeros((PART, PART), np.float16)
    wc0[0, 0] = 1.0
    wc127 = np.zeros((PART, PART), np.float16)
    wc127[127, 127] = 1.0

    in_maps = []
    for c in range(NCORES):
        m = np.zeros((PART, NCORES, RP * 2 * GW), np.float16)
        sel = np.zeros((NCORES, RP, 2 * GW), np.float16)
        if c > 0:
            sel[c - 1, :, GW:2 * GW] = 1.0   # left neighbor's right-send
        if c < NCORES - 1:
            sel[c + 1, :, 0:GW] = 1.0        # right neighbor's left-send
        m[:, :, :] = sel.reshape(NCORES, RP * 2 * GW)[None, :, :]
        in_maps.append({
            "bxdT": _tile_of(bxd, c),
            "bxuT": _tile_of(bxu, c),
            "bydT": _tile_of(byd, c),
            "byuT": _tile_of(byu, c),
            "cpT": _tile_of(cp, c),
            "rsT": _tile_of(rs, c, np.float32)[:, :, GW:GW + MC].copy(),
            "mask": m,
            "wI": ident, "wUp": wup, "wDn": wdn,
            "wC0": wc0, "wC127": wc127,
        })
    return in_maps


def _build():
    import concourse.bass as bass
    import concourse.bacc as bacc
    import concourse.mybir as mybir
    from concourse.tile import TileContext

    f32 = mybir.dt.float32
    f16 = mybir.dt.float16
    AF = mybir.ActivationFunctionType
    OP = mybir.AluOpType
    SG = RP * 2 * GW                     # send payload elems per partition

    nc = bacc.Bacc("TRN2", target_bir_lowering=False, debug=False,
                   num_devices=NCORES)
    dp = nc.declare_dram_parameter
    bxd_d = dp("bxdT", [PART, RP, CW], f16, isOutput=False)
    bxu_d = dp("bxuT", [PART, RP, CW], f16, isOutput=False)
    byd_d = dp("bydT", [PART, RP, CW], f16, isOutput=False)
    byu_d = dp("byuT", [PART, RP, CW], f16, isOutput=False)
    cp_d = dp("cpT", [PART, RP, CW], f16, isOutput=False)
    rs_d = dp("rsT", [PART, RP, MC], f32, isOutput=False)
    mask_d = dp("mask", [PART, NCORES, SG], f16, isOutput=False)
    w_ds = {nm: dp(nm, [PART, PART], f16, isOutput=False)
            for nm in ("wI", "wUp", "wDn", "wC0", "wC127")}
    pout_d = dp("pout", [PART, RP * MC], f32, isOutput=True)

    with TileContext(nc) as tc:
        with (
            tc.tile_pool(name="coef", bufs=1) as coef,
            tc.tile_pool(name="work", bufs=2) as work,
            tc.tile_pool(name="qp", bufs=2, space="PSUM") as qp,
            tc.tile_pool(name="dramp", bufs=2, space="DRAM") as dramp,
        ):
            bxdT = coef.tile([PART, RP, CW], f16, name="bxdT_t")
            bxuT = coef.tile([PART, RP, CW], f16, name="bxuT_t")
            bydT = coef.tile([PART, RP, CW], f16, name="bydT_t")
            byuT = coef.tile([PART, RP, CW], f16, name="byuT_t")
            cpT = coef.tile([PART, RP, CW], f16, name="cpT_t")
            rsT = coef.tile([PART, RP, MC], f32, name="rsT_t")
            mask = coef.tile([PART, NCORES, SG], f16, name="mask_t")
            ws = {nm: coef.tile([PART, PART], f16, name=f"{nm}_t")
                  for nm in w_ds}
            nc.sync.dma_start(out=bxdT[:, :, :], in_=bxd_d[:, :, :])
            nc.sync.dma_start(out=bxuT[:, :, :], in_=bxu_d[:, :, :])
            nc.sync.dma_start(out=bydT[:, :, :], in_=byd_d[:, :, :])
            nc.sync.dma_start(out=byuT[:, :, :], in_=byu_d[:, :, :])
            nc.sync.dma_start(out=cpT[:, :, :], in_=cp_d[:, :, :])
            nc.sync.dma_start(out=rsT[:, :, :], in_=rs_d[:, :, :])
            nc.sync.dma_start(out=mask[:, :, :], in_=mask_d[:, :, :])
            for nm, d in w_ds.items():
                nc.sync.dma_start(out=ws[nm][:, :], in_=d[:, :])

            cpF = cpT.rearrange("p r c -> p (r c)")
            V = nc.vector
            mm = nc.tensor.matmul

            q = work.tile([PART, RP, CW], f16, tag="q", name="q_0")
            V.memset(q[:, :, :], 0.0)

            for t in range(1, TS + 1):
                A = work.tile([PART, RP, CW], f16, tag="A", name=f"A_{t}")
                B = work.tile([PART, RP, CW], f16, tag="B", name=f"B_{t}")
                C1 = work.tile([PART, RP, CW], f16, tag="C1", name=f"C1_{t}")
                D1 = work.tile([PART, RP, CW], f16, tag="D1", name=f"D1_{t}")
                V.tensor_mul(A[:, :, :], bxdT[:, :, :], q[:, :, :])
                V.tensor_mul(B[:, :, :], bxuT[:, :, :], q[:, :, :])
                V.tensor_mul(C1[:, :, :], bydT[:, :, :], q[:, :, :])
                V.tensor_mul(D1[:, :, :], byuT[:, :, :], q[:, :, :])
                Af = A.rearrange("p r c -> p (r c)")
                Bf = B.rearrange("p r c -> p (r c)")
                Cf = C1.rearrange("p r c -> p (r c)")
                Df = D1.rearrange("p r c -> p (r c)")

                ps = qp.tile([PART, 1536], f32, tag="ps", name=f"ps_{t}")
                # Emit matmuls in global dependency order (PE executes its
                # queue in order; grouping by bank would stall PE on the last
                # product once per bank).  Per-bank accumulation groups:
                # cp opens (start=True), y-down closes (stop=True).
                for lo, hi in BANKS:
                    mm(ps[:, lo:hi], ws["wI"], cpF[:, lo:hi],
                       start=True, stop=False)
                for lo, hi in BANKS:
                    # x-up: out x gets A[x+CW], valid out < 7*CW
                    a, b = lo, min(hi, 7 * CW)
                    if a < b:
                        mm(ps[:, a:b], ws["wI"], Af[:, a + CW:b + CW],
                           start=False, stop=False)
                for lo, hi in BANKS:
                    # x-down: out x gets B[x-CW], valid out >= CW
                    a, b = max(lo, CW), hi
                    if a < b:
                        mm(ps[:, a:b], ws["wI"], Bf[:, a - CW:b - CW],
                           start=False, stop=False)
                # cross-partition row couplings (need A, B only)
                mm(ps[:, 0:CW], ws["wDn"], Bf[:, 7 * CW:F],
                   start=False, stop=False)
                mm(ps[:, 0:CW], ws["wC0"], Af[:, 0:CW],
                   start=False, stop=False)
                mm(ps[:, 7 * CW:F], ws["wUp"], Af[:, 0:CW],
                   start=False, stop=False)
                mm(ps[:, 7 * CW:F], ws["wC127"], Bf[:, 7 * CW:F],
                   start=False, stop=False)
                for lo, hi in BANKS:
                    # y-up: out x gets C1[x+1], valid out < F-1
                    a, b = lo, min(hi, F - 1)
                    if a < b:
                        mm(ps[:, a:b], ws["wI"], Cf[:, a + 1:b + 1],
                           start=False, stop=False)
                for lo, hi in BANKS:
                    # y-down closes each bank's accumulation group
                    a, b = max(lo, 1), hi
                    mm(ps[:, a:b], ws["wI"], Df[:, a - 1:b - 1],
                       start=False, stop=True)

                qn = work.tile([PART, RP, CW], f16, tag="q", name=f"q_{t}")
                qnF = qn.rearrange("p r c -> p (r c)")
                # bank-sliced eviction: ACT takes banks 0,1 while PE finishes
                # bank 2, whose evict runs on the (idle) DVE in parallel
                for lo, hi in BANKS[:2]:
                    nc.scalar.activation(qnF[:, lo:hi], ps[:, lo:hi], AF.Copy)
                V.tensor_copy(qnF[:, BANKS[2][0]:BANKS[2][1]],
                              ps[:, BANKS[2][0]:BANKS[2][1]])
                q = qn

                if t % GW == 0 and t < TS:
                    stage = work.tile([PART, RP, 2 * GW], f16, tag="stage",
                                      name=f"stage_{t}")
                    V.tensor_copy(stage[:, :, 0:GW], q[:, :, GW:2 * GW])
                    V.tensor_copy(stage[:, :, GW:2 * GW], q[:, :, MC:MC + GW])
                    bounce = dramp.tile([PART, SG], f16, tag="bounce",
                                        name=f"bounce_{t}")
                    gath = dramp.tile([NCORES, PART, SG], f16, tag="gath",
                                      addr_space="Shared", name=f"gath_{t}")
                    stF = stage.rearrange("p r c -> p (r c)")
                    nc.sync.dma_start(out=bounce[:, :], in_=stF[:, :])
                    nc.gpsimd.collective_compute(
                        "AllGather", OP.bypass,
                        ins=[bounce.opt()], outs=[gath.opt()],
                        replica_groups=[list(range(NCORES))],
                    )
                    GG = work.tile([PART, NCORES, SG], f16, tag="GG",
                                   name=f"GG_{t}")
                    nc.sync.dma_start(out=GG[:, :, :],
                                      in_=gath[:, :, :].transpose([1, 0, 2]))
                    GGm = work.tile([PART, NCORES, SG], f16, tag="GGm",
                                    name=f"GGm_{t}")
                    V.tensor_mul(GGm[:, :, :], GG[:, :, :], mask[:, :, :])
                    T1 = work.tile([PART, 4, SG], f16, tag="T1", name=f"T1_{t}")
                    V.tensor_add(T1[:, :, :], GGm[:, 0:4, :], GGm[:, 4:8, :])
                    T2 = work.tile([PART, 2, SG], f16, tag="T2", name=f"T2_{t}")
                    V.tensor_add(T2[:, :, :], T1[:, 0:2, :], T1[:, 2:4, :])
                    R = work.tile([PART, 1, SG], f16, tag="R", name=f"R_{t}")
                    V.tensor_add(R[:, :, :], T2[:, 0:1, :], T2[:, 1:2, :])
                    Rv = R.rearrange("p a (r c) -> p (a r) c", c=2 * GW)
                    V.tensor_copy(q[:, :, 0:GW], Rv[:, :, GW:2 * GW])
                    V.tensor_copy(q[:, :, MC + GW:CW], Rv[:, :, 0:GW])

            outt = coef.tile([PART, RP, MC], f32, name="outt")
            V.tensor_mul(outt[:, :, :], q[:, :, GW:GW + MC], rsT[:, :, :])
            oF = outt.rearrange("p r c -> p (r c)")
            nc.sync.dma_start(out=pout_d[:, :], in_=oF[:, :])

    nc.finalize()
    return nc


def _get_nc():
    if "nc" not in _cached:
        _cached["nc"] = _build()
    return _cached["nc"]


def kernel(u, f, time_steps):
    from concourse.bass_utils import run_bass_kernel_spmd

    u = np.asarray(u)
    f = np.asarray(f)
    assert int(time_steps) == TS and u.shape == (N, N)
    nc = _get_nc()
    in_maps = _host_inputs(u, f)
    res = run_bass_kernel_spmd(nc, in_maps, list(range(NCORES))).results
    h = 1.0 / (N - 1)
    xs = (np.arange(N, dtype=np.float64) * h).astype(np.float32)
    out = np.empty((N, N), dtype=np.float32)
    for c in range(NCORES):
        blk = res[c]["pout"].reshape(PART, RP, MC).reshape(N, MC)
        out[:, MC * c:MC * (c + 1)] = blk
    out[:, 0] = xs
    out[:, N - 1] = 1.0 - xs
    return out
